# revision 9
# baseline (speedup 1.0000x reference)
"""Trainium2 Bass kernel for nn_Cat_20916490732013.

Computes out = log_softmax(logits, axis=-1) where
    L = start_emb @ proj                  # [n, f]
    R = output_emb @ proj                 # [c, f]
    logits = log(exp(L) @ exp(R).T) / temp

Sharding: column-parallel over the class axis (8 cores, 4096 classes each).
start_emb and proj are replicated; output_emb and the output are sharded.

Key algebraic identity used to avoid a second pass over the [n, c] logits:
    logsumexp_j(logits_i) = log( sum_j sum_f exp(L_if) exp(R_jf) )
                          = log( sum_f exp(L_if) * V_f ),   V_f = sum_j exp(R_jf)
so each core computes a per-row partial Z_i^(c) = sum_f exp(L_if) * v_f^(c)
(v from its own class shard) and a single 16 KiB AllReduce(add) of Z yields the
global normalizer. The output tile epilogue is then one scalar-engine op:
    out_ij = Ln(S_ij * (1/Z_i)),  S = exp(L) @ exp(R).T
(The input distribution keeps |L|,|R| < ~1, so unstabilized exp is exact-safe;
the per-row max terms cancel identically in log-softmax.)
"""

import sys

for _p in ("/opt/trn_rl_repo",):
    if _p not in sys.path:
        sys.path.insert(0, _p)

from contextlib import ExitStack

import numpy as np

N_CORES = 8
N_STARTS = 4096
N_CLASSES = 32768
EMB = 128
FEAT = 256
SHARD = N_CLASSES // N_CORES  # 4096

_CACHE = {}


def build_bass(n=N_STARTS, shard=SHARD, feat=FEAT, emb=EMB, n_cores=N_CORES,
               use_bf16=True):
    """Build + compile the per-core Bass program (SPMD: same program on all cores)."""
    import concourse.tile as tile
    from concourse import bacc, mybir
    from concourse.masks import make_identity

    f32 = mybir.dt.float32
    bf16 = mybir.dt.bfloat16
    mm_dt = bf16 if use_bf16 else f32
    P = 128
    assert emb == P and feat % P == 0 and n % P == 0 and shard % P == 0
    FB = feat // P            # feature blocks on the partition axis (2)
    NT = n // P               # output row tiles (32)
    JT = shard // P           # class tiles per core (32)
    NCH = 512                 # matmul free-dim per instruction (fp32 max)
    JW = min(4 * NCH, shard)  # psum group width for the main matmul (2048)
    JH = shard // JW          # psum groups per row tile

    Exp = mybir.ActivationFunctionType.Exp
    Ln = mybir.ActivationFunctionType.Ln
    X = mybir.AxisListType.X

    nc = bacc.Bacc("TRN2", target_bir_lowering=False, debug=False,
                   num_devices=n_cores)
    se_d = nc.dram_tensor("se", [n, emb], f32, kind="ExternalInput").ap()
    oe_d = nc.dram_tensor("oe", [shard, emb], f32, kind="ExternalInput").ap()
    pj_d = nc.dram_tensor("proj", [emb, feat], f32, kind="ExternalInput").ap()
    out_d = nc.dram_tensor("out", [n, shard], f32, kind="ExternalOutput").ap()

    with ExitStack() as ctx:
        tc = ctx.enter_context(tile.TileContext(nc))
        const = ctx.enter_context(tc.tile_pool(name="const", bufs=1))
        big = ctx.enter_context(tc.tile_pool(name="big", bufs=1))
        obp = ctx.enter_context(tc.tile_pool(name="obp", bufs=3))
        dram = ctx.enter_context(tc.tile_pool(name="dram", bufs=1, space="DRAM"))

        ident = const.tile([P, P], f32, name="ident")
        make_identity(nc, ident)
        proj_sb = const.tile([P, feat], f32, name="proj_sb")
        nc.sync.dma_start(out=proj_sb, in_=pj_d)

        # Inputs in natural layout: partition = row-within-tile.
        se_sb = big.tile([P, NT, emb], f32, name="se_sb")
        oe_sb = big.tile([P, JT, emb], f32, name="oe_sb")
        se_r = se_d.rearrange("(t p) k -> p t k", p=P)
        oe_r = oe_d.rearrange("(t p) k -> p t k", p=P)
        # Chunked loads so transposes can start before the full tensor lands.
        ldc = 8
        for g in range(0, NT, ldc):
            ge = min(g + ldc, NT)
            nc.sync.dma_start(out=se_sb[:, g:ge, :], in_=se_r[:, g:ge, :])
        for g in range(0, JT, ldc):
            ge = min(g + ldc, JT)
            nc.sync.dma_start(out=oe_sb[:, g:ge, :], in_=oe_r[:, g:ge, :])

        # Transposed inputs: [k, n] / [k, j] — contraction dim on partitions.
        seT = big.tile([P, n], f32, name="seT")
        oeT = big.tile([P, shard], f32, name="oeT")
        # exp of projections, transposed: [f, n] / [f, j], split into FB blocks.
        # elT kept in fp32 for the precise normalizer matvec; a bf16 copy
        # (elT16/erT16) feeds the big matmul at full PE streaming rate.
        elT = [big.tile([P, n], f32, name=f"elT{fb}") for fb in range(FB)]
        elTm = ([big.tile([P, n], mm_dt, name=f"elTm{fb}") for fb in range(FB)]
                if use_bf16 else elT)
        erTm = [big.tile([P, shard], mm_dt, name=f"erTm{fb}") for fb in range(FB)]
        vown = const.tile([P, FB], f32, name="vown")
        z_sb = const.tile([P, NT], f32, name="z_sb")
        zgath = const.tile([P, NT, n_cores], f32, name="zgath")
        Z_sb = const.tile([P, NT], f32, name="Z_sb")
        logz = const.tile([P, NT], f32, name="logz")

        # ---- Phase 1: transpose inputs, project, exp, partial normalizer ----
        # OE side first so the main matmul's rhs (erTm) is ready earliest;
        # copies alternate Vector/Scalar to keep pace with PE transposes.
        with tc.tile_pool(name="ps_tr", bufs=3, space="PSUM") as ps_tr, \
             tc.tile_pool(name="ps_pj", bufs=3, space="PSUM") as ps_pj, \
             tc.tile_pool(name="ps_z", bufs=2, space="PSUM") as ps_z:
            for src, dstT, tcnt in ((oe_sb, oeT, JT), (se_sb, seT, NT)):
                for t in range(tcnt):
                    pst = ps_tr.tile([P, P], f32, name="pst", tag="pst")
                    nc.tensor.transpose(pst, src[:, t, :], ident)
                    if t % 2 == 0:
                        nc.vector.tensor_copy(out=dstT[:, t * P:(t + 1) * P], in_=pst)
                    else:
                        nc.scalar.copy(out=dstT[:, t * P:(t + 1) * P], in_=pst)

            # LT/RT = projT-block @ (OE.T / SE.T); exp applied on PSUM->SBUF.
            for srcT, width, kind in ((oeT, shard, "r"), (seT, n, "l")):
                for fb in range(FB):
                    pw = proj_sb[:, fb * P:(fb + 1) * P]
                    dst = erTm[fb] if kind == "r" else elT[fb]
                    for c0 in range(0, width, NCH):
                        w = min(NCH, width - c0)
                        psl = ps_pj.tile([P, NCH], f32, name="psl", tag="psl")
                        nc.tensor.matmul(psl[:, :w], pw, srcT[:, c0:c0 + w],
                                         start=True, stop=True)
                        nc.scalar.activation(out=dst[:, c0:c0 + w], in_=psl[:, :w],
                                             func=Exp)
                if kind == "r":
                    # v_f = sum over this core's classes of exp(R_jf)
                    for fb in range(FB):
                        nc.vector.reduce_sum(out=vown[:, fb:fb + 1], in_=erTm[fb],
                                             axis=X)

            # bf16 copy of exp(L).T for the main matmul, chunked so the main
            # loop can start before the whole cast is done.
            if use_bf16:
                for fb in range(FB):
                    for c0 in range(0, n, 1024):
                        w = min(1024, n - c0)
                        nc.vector.tensor_copy(out=elTm[fb][:, c0:c0 + w],
                                              in_=elT[fb][:, c0:c0 + w])

            # Per-row partial normalizer z_i = sum_f exp(L_if) * v_f  -> [n]
            for t in range(NT):
                psz = ps_z.tile([P, 1], f32, name="psz", tag="psz")
                for fb in range(FB):
                    nc.tensor.matmul(psz, elT[fb][:, t * P:(t + 1) * P],
                                     vown[:, fb:fb + 1],
                                     start=(fb == 0), stop=(fb == FB - 1))
                nc.scalar.copy(out=z_sb[:, t:t + 1], in_=psz)

        # ---- AllGather per-core partial normalizers (16 KiB), reduce locally
        # (AllGather has a ~2x lower latency floor than AllReduce).
        cc_in = dram.tile([P, NT], f32, name="cc_in")
        cc_out = dram.tile([P * n_cores, NT], f32, name="cc_out")
        nc.sync.dma_start(out=cc_in, in_=z_sb)
        nc.gpsimd.collective_compute(
            "AllGather", mybir.AluOpType.bypass,
            replica_groups=[list(range(n_cores))],
            ins=[cc_in.opt()], outs=[cc_out.opt()],
        )
        # cc_out rows are [core c][partition p]; bring back as [p, t, c].
        nc.sync.dma_start(out=zgath, in_=cc_out.rearrange("(c p) t -> p t c", p=P))
        nc.vector.reduce_sum(out=Z_sb, in_=zgath, axis=X)
        nc.scalar.activation(out=logz, in_=Z_sb, func=Ln)

        # ---- Phase 2: S = exp(L) @ exp(R).T, out = Ln(S) - logZ, store ----
        # The logZ subtraction runs on the (otherwise idle) Vector engine so
        # neither PE nor ACT ever waits on the collective.
        with tc.tile_pool(name="ps_mm", bufs=2, space="PSUM") as ps_mm:
            for t in range(NT):
                ob = obp.tile([P, shard], f32, name="ob", tag="ob")
                for jh in range(JH):
                    ps = ps_mm.tile([P, JW], f32, name="ps", tag="ps")
                    for fb in range(FB):
                        lw = elTm[fb][:, t * P:(t + 1) * P]
                        for c0 in range(0, JW, NCH):
                            nc.tensor.matmul(
                                ps[:, c0:c0 + NCH], lw,
                                erTm[fb][:, jh * JW + c0: jh * JW + c0 + NCH],
                                start=(fb == 0), stop=(fb == FB - 1))
                    osl = ob[:, jh * JW:(jh + 1) * JW]
                    nc.scalar.activation(out=osl, in_=ps, func=Ln)
                    nc.vector.tensor_scalar_sub(osl, osl, logz[:, t:t + 1])
                nc.sync.dma_start(out=out_d[t * P:(t + 1) * P, :], in_=ob)

    nc.compile()
    return nc


def _get_nc():
    if "nc" not in _CACHE:
        _CACHE["nc"] = build_bass()
    return _CACHE["nc"]


def _numpy_fallback(start_emb, output_emb, proj, temp):
    L = start_emb.astype(np.float64) @ proj.astype(np.float64)
    R = output_emb.astype(np.float64) @ proj.astype(np.float64)
    mL = L.max(-1, keepdims=True)
    mR = R.max(-1, keepdims=True)
    S = np.exp(L - mL) @ np.exp(R - mR).T
    logits = (np.log(S) + mL + mR.T) / float(temp)
    m = logits.max(-1, keepdims=True)
    out = logits - m - np.log(np.exp(logits - m).sum(-1, keepdims=True))
    return out.astype(np.float32)


def run_on_hw(start_emb, output_emb, proj, trace=False, **trace_kwargs):
    from concourse.bass_utils import run_bass_kernel_spmd

    nc = _get_nc()
    se = np.ascontiguousarray(start_emb, dtype=np.float32)
    oe = np.ascontiguousarray(output_emb, dtype=np.float32)
    pj = np.ascontiguousarray(proj, dtype=np.float32)
    in_maps = [
        {"se": se, "oe": oe[c * SHARD:(c + 1) * SHARD], "proj": pj}
        for c in range(N_CORES)
    ]
    res = run_bass_kernel_spmd(nc, in_maps, core_ids=list(range(N_CORES)),
                               trace=trace, **trace_kwargs)
    out = np.concatenate([res.results[c]["out"] for c in range(N_CORES)], axis=1)
    return out, res


def kernel(start_emb, output_emb, proj, temp):
    t = float(np.asarray(temp).reshape(-1)[0])
    if t != 1.0:
        return _numpy_fallback(np.asarray(start_emb), np.asarray(output_emb),
                               np.asarray(proj), t)
    out, _ = run_on_hw(start_emb, output_emb, proj, trace=False)
    return out


# revision 12
# speedup vs baseline: 1.1883x; 1.1883x over previous
"""Trainium2 Bass kernel for nn_Cat_20916490732013.

Computes out = log_softmax(logits, axis=-1) where
    L = start_emb @ proj                  # [n, f]
    R = output_emb @ proj                 # [c, f]
    logits = log(exp(L) @ exp(R).T) / temp

Sharding: column-parallel over the class axis (8 cores, 4096 classes each).
start_emb and proj are replicated; output_emb and the output are sharded.

Key algebraic identity used to avoid a second pass over the [n, c] logits:
    logsumexp_j(logits_i) = log( sum_j sum_f exp(L_if) exp(R_jf) )
                          = log( sum_f exp(L_if) * V_f ),   V_f = sum_j exp(R_jf)
so each core computes a per-row partial Z_i^(c) = sum_f exp(L_if) * v_f^(c)
(v from its own class shard) and a single 16 KiB AllReduce(add) of Z yields the
global normalizer. The output tile epilogue is then one scalar-engine op:
    out_ij = Ln(S_ij * (1/Z_i)),  S = exp(L) @ exp(R).T
(The input distribution keeps |L|,|R| < ~1, so unstabilized exp is exact-safe;
the per-row max terms cancel identically in log-softmax.)
"""

import sys

for _p in ("/opt/trn_rl_repo",):
    if _p not in sys.path:
        sys.path.insert(0, _p)

from contextlib import ExitStack

import numpy as np

N_CORES = 8
N_STARTS = 4096
N_CLASSES = 32768
EMB = 128
FEAT = 256
SHARD = N_CLASSES // N_CORES  # 4096

_CACHE = {}


def build_bass(n=N_STARTS, shard=SHARD, feat=FEAT, emb=EMB, n_cores=N_CORES,
               use_bf16=True):
    """Build + compile the per-core Bass program (SPMD: same program on all cores)."""
    import concourse.tile as tile
    from concourse import bacc, mybir
    from concourse.masks import make_identity

    f32 = mybir.dt.float32
    bf16 = mybir.dt.bfloat16
    mm_dt = bf16 if use_bf16 else f32
    P = 128
    assert emb == P and feat % P == 0 and n % P == 0 and shard % P == 0
    FB = feat // P            # feature blocks on the partition axis (2)
    NT = n // P               # output row tiles (32)
    JT = shard // P           # class tiles per core (32)
    NCH = 512                 # matmul free-dim per instruction (fp32 max)
    JW = min(4 * NCH, shard)  # psum group width for the main matmul (2048)
    JH = shard // JW          # psum groups per row tile

    Exp = mybir.ActivationFunctionType.Exp
    Ln = mybir.ActivationFunctionType.Ln
    X = mybir.AxisListType.X

    nc = bacc.Bacc("TRN2", target_bir_lowering=False, debug=False,
                   num_devices=n_cores)
    se_d = nc.dram_tensor("se", [n, emb], f32, kind="ExternalInput").ap()
    oe_d = nc.dram_tensor("oe", [shard, emb], f32, kind="ExternalInput").ap()
    pj_d = nc.dram_tensor("proj", [emb, feat], f32, kind="ExternalInput").ap()
    out_d = nc.dram_tensor("out", [n, shard], f32, kind="ExternalOutput").ap()

    with ExitStack() as ctx:
        tc = ctx.enter_context(tile.TileContext(nc))
        const = ctx.enter_context(tc.tile_pool(name="const", bufs=1))
        big = ctx.enter_context(tc.tile_pool(name="big", bufs=1))
        obp = ctx.enter_context(tc.tile_pool(name="obp", bufs=3))
        dram = ctx.enter_context(tc.tile_pool(name="dram", bufs=1, space="DRAM"))

        ident = const.tile([P, P], f32, name="ident")
        make_identity(nc, ident)
        proj_sb = const.tile([P, feat], f32, name="proj_sb")
        nc.sync.dma_start(out=proj_sb, in_=pj_d)

        # Inputs in natural layout: partition = row-within-tile.
        se_sb = big.tile([P, NT, emb], f32, name="se_sb")
        oe_sb = big.tile([P, JT, emb], f32, name="oe_sb")
        se_r = se_d.rearrange("(t p) k -> p t k", p=P)
        oe_r = oe_d.rearrange("(t p) k -> p t k", p=P)
        # Chunked loads so transposes can start before the full tensor lands.
        ldc = 8
        for g in range(0, NT, ldc):
            ge = min(g + ldc, NT)
            nc.sync.dma_start(out=se_sb[:, g:ge, :], in_=se_r[:, g:ge, :])
        for g in range(0, JT, ldc):
            ge = min(g + ldc, JT)
            nc.sync.dma_start(out=oe_sb[:, g:ge, :], in_=oe_r[:, g:ge, :])

        # Transposed inputs: [k, n] / [k, j] — contraction dim on partitions.
        seT = big.tile([P, n], f32, name="seT")
        oeT = big.tile([P, shard], f32, name="oeT")
        # exp of projections, transposed: [f, n] / [f, j], split into FB blocks.
        # elT kept in fp32 for the precise normalizer matvec; a bf16 copy
        # (elT16/erT16) feeds the big matmul at full PE streaming rate.
        elT = [big.tile([P, n], f32, name=f"elT{fb}") for fb in range(FB)]
        elTm = ([big.tile([P, n], mm_dt, name=f"elTm{fb}") for fb in range(FB)]
                if use_bf16 else elT)
        erTm = [big.tile([P, shard], mm_dt, name=f"erTm{fb}") for fb in range(FB)]
        vown = const.tile([P, FB], f32, name="vown")
        z_sb = const.tile([P, NT], f32, name="z_sb")
        zgath = const.tile([P, n_cores, NT], f32, name="zgath")
        Z_sb = const.tile([P, NT], f32, name="Z_sb")
        logz = const.tile([P, NT], f32, name="logz")

        # ---- Phase 1: transpose inputs, project, exp, partial normalizer ----
        # OE side first so the main matmul's rhs (erTm) is ready earliest;
        # copies alternate Vector/Scalar to keep pace with PE transposes.
        with tc.tile_pool(name="ps_tr", bufs=3, space="PSUM") as ps_tr, \
             tc.tile_pool(name="ps_pj", bufs=3, space="PSUM") as ps_pj, \
             tc.tile_pool(name="ps_z", bufs=2, space="PSUM") as ps_z:
            for src, dstT, tcnt in ((oe_sb, oeT, JT), (se_sb, seT, NT)):
                for t in range(tcnt):
                    pst = ps_tr.tile([P, P], f32, name="pst", tag="pst")
                    nc.tensor.transpose(pst, src[:, t, :], ident)
                    if t % 2 == 0:
                        nc.vector.tensor_copy(out=dstT[:, t * P:(t + 1) * P], in_=pst)
                    else:
                        nc.scalar.copy(out=dstT[:, t * P:(t + 1) * P], in_=pst)

            # LT/RT = projT-block @ (OE.T / SE.T); exp applied on PSUM->SBUF.
            for srcT, width, kind in ((oeT, shard, "r"), (seT, n, "l")):
                for fb in range(FB):
                    pw = proj_sb[:, fb * P:(fb + 1) * P]
                    dst = erTm[fb] if kind == "r" else elT[fb]
                    for c0 in range(0, width, NCH):
                        w = min(NCH, width - c0)
                        psl = ps_pj.tile([P, NCH], f32, name="psl", tag="psl")
                        nc.tensor.matmul(psl[:, :w], pw, srcT[:, c0:c0 + w],
                                         start=True, stop=True)
                        nc.scalar.activation(out=dst[:, c0:c0 + w], in_=psl[:, :w],
                                             func=Exp)
                if kind == "r":
                    # v_f = sum over this core's classes of exp(R_jf)
                    for fb in range(FB):
                        nc.vector.reduce_sum(out=vown[:, fb:fb + 1], in_=erTm[fb],
                                             axis=X)

            # bf16 copy of exp(L).T for the main matmul, chunked so the main
            # loop can start before the whole cast is done.
            if use_bf16:
                for fb in range(FB):
                    for c0 in range(0, n, 1024):
                        w = min(1024, n - c0)
                        nc.vector.tensor_copy(out=elTm[fb][:, c0:c0 + w],
                                              in_=elT[fb][:, c0:c0 + w])

            # Per-row partial normalizer z_i = sum_f exp(L_if) * v_f  -> [n]
            for t in range(NT):
                psz = ps_z.tile([P, 1], f32, name="psz", tag="psz")
                for fb in range(FB):
                    nc.tensor.matmul(psz, elT[fb][:, t * P:(t + 1) * P],
                                     vown[:, fb:fb + 1],
                                     start=(fb == 0), stop=(fb == FB - 1))
                nc.scalar.copy(out=z_sb[:, t:t + 1], in_=psz)

        # ---- AllGather per-core partial normalizers (16 KiB), reduce locally
        # (AllGather has a ~2x lower latency floor than AllReduce).
        cc_in = dram.tile([P, NT], f32, name="cc_in")
        cc_out = dram.tile([P * n_cores, NT], f32, name="cc_out")
        nc.sync.dma_start(out=cc_in, in_=z_sb)
        nc.gpsimd.collective_compute(
            "AllGather", mybir.AluOpType.bypass,
            replica_groups=[list(range(n_cores))],
            ins=[cc_in.opt()], outs=[cc_out.opt()],
        )
        # cc_out rows are [core c][partition p]; bring back as [p, c, t] so the
        # innermost DMA dim stays contiguous, then sum the 8 core blocks.
        nc.sync.dma_start(out=zgath, in_=cc_out.rearrange("(c p) t -> p c t", p=P))
        nc.vector.tensor_add(Z_sb, zgath[:, 0, :], zgath[:, 1, :])
        for c in range(2, n_cores):
            nc.vector.tensor_add(Z_sb, Z_sb, zgath[:, c, :])
        nc.scalar.activation(out=logz, in_=Z_sb, func=Ln)

        # ---- Phase 2: S = exp(L) @ exp(R).T, out = Ln(S) - logZ, store ----
        # The logZ subtraction runs on the (otherwise idle) Vector engine so
        # neither PE nor ACT ever waits on the collective.
        with tc.tile_pool(name="ps_mm", bufs=2, space="PSUM") as ps_mm:
            for t in range(NT):
                ob = obp.tile([P, shard], f32, name="ob", tag="ob")
                for jh in range(JH):
                    ps = ps_mm.tile([P, JW], f32, name="ps", tag="ps")
                    for fb in range(FB):
                        lw = elTm[fb][:, t * P:(t + 1) * P]
                        for c0 in range(0, JW, NCH):
                            nc.tensor.matmul(
                                ps[:, c0:c0 + NCH], lw,
                                erTm[fb][:, jh * JW + c0: jh * JW + c0 + NCH],
                                start=(fb == 0), stop=(fb == FB - 1))
                    osl = ob[:, jh * JW:(jh + 1) * JW]
                    nc.scalar.activation(out=osl, in_=ps, func=Ln)
                    nc.vector.tensor_scalar_sub(osl, osl, logz[:, t:t + 1])
                nc.sync.dma_start(out=out_d[t * P:(t + 1) * P, :], in_=ob)

    nc.compile()
    return nc


def _get_nc():
    if "nc" not in _CACHE:
        _CACHE["nc"] = build_bass()
    return _CACHE["nc"]


def _numpy_fallback(start_emb, output_emb, proj, temp):
    L = start_emb.astype(np.float64) @ proj.astype(np.float64)
    R = output_emb.astype(np.float64) @ proj.astype(np.float64)
    mL = L.max(-1, keepdims=True)
    mR = R.max(-1, keepdims=True)
    S = np.exp(L - mL) @ np.exp(R - mR).T
    logits = (np.log(S) + mL + mR.T) / float(temp)
    m = logits.max(-1, keepdims=True)
    out = logits - m - np.log(np.exp(logits - m).sum(-1, keepdims=True))
    return out.astype(np.float32)


def run_on_hw(start_emb, output_emb, proj, trace=False, **trace_kwargs):
    from concourse.bass_utils import run_bass_kernel_spmd

    nc = _get_nc()
    se = np.ascontiguousarray(start_emb, dtype=np.float32)
    oe = np.ascontiguousarray(output_emb, dtype=np.float32)
    pj = np.ascontiguousarray(proj, dtype=np.float32)
    in_maps = [
        {"se": se, "oe": oe[c * SHARD:(c + 1) * SHARD], "proj": pj}
        for c in range(N_CORES)
    ]
    res = run_bass_kernel_spmd(nc, in_maps, core_ids=list(range(N_CORES)),
                               trace=trace, **trace_kwargs)
    out = np.concatenate([res.results[c]["out"] for c in range(N_CORES)], axis=1)
    return out, res


def kernel(start_emb, output_emb, proj, temp):
    t = float(np.asarray(temp).reshape(-1)[0])
    if t != 1.0:
        return _numpy_fallback(np.asarray(start_emb), np.asarray(output_emb),
                               np.asarray(proj), t)
    out, _ = run_on_hw(start_emb, output_emb, proj, trace=False)
    return out


# revision 15
# speedup vs baseline: 1.2850x; 1.0814x over previous
"""Trainium2 Bass kernel for nn_Cat_20916490732013.

Computes out = log_softmax(logits, axis=-1) where
    L = start_emb @ proj                  # [n, f]
    R = output_emb @ proj                 # [c, f]
    logits = log(exp(L) @ exp(R).T) / temp

Sharding: column-parallel over the class axis (8 cores, 4096 classes each).
start_emb and proj are replicated; output_emb and the output are sharded.

Key algebraic identity used to avoid a second pass over the [n, c] logits:
    logsumexp_j(logits_i) = log( sum_j sum_f exp(L_if) exp(R_jf) )
                          = log( sum_f exp(L_if) * V_f ),   V_f = sum_j exp(R_jf)
so each core computes a per-row partial Z_i^(c) = sum_f exp(L_if) * v_f^(c)
(v from its own class shard) and a single 16 KiB AllReduce(add) of Z yields the
global normalizer. The output tile epilogue is then one scalar-engine op:
    out_ij = Ln(S_ij * (1/Z_i)),  S = exp(L) @ exp(R).T
(The input distribution keeps |L|,|R| < ~1, so unstabilized exp is exact-safe;
the per-row max terms cancel identically in log-softmax.)
"""

import sys

for _p in ("/opt/trn_rl_repo",):
    if _p not in sys.path:
        sys.path.insert(0, _p)

from contextlib import ExitStack

import numpy as np

N_CORES = 8
N_STARTS = 4096
N_CLASSES = 32768
EMB = 128
FEAT = 256
SHARD = N_CLASSES // N_CORES  # 4096

_CACHE = {}


def build_bass(n=N_STARTS, shard=SHARD, feat=FEAT, emb=EMB, n_cores=N_CORES,
               use_bf16=True):
    """Build + compile the per-core Bass program (SPMD: same program on all cores)."""
    import concourse.tile as tile
    from concourse import bacc, mybir

    f32 = mybir.dt.float32
    bf16 = mybir.dt.bfloat16
    mm_dt = bf16 if use_bf16 else f32
    P = 128
    assert emb == P and feat % P == 0 and n % P == 0 and shard % P == 0
    FB = feat // P            # feature blocks on the partition axis (2)
    NT = n // P               # output row tiles (32)
    JT = shard // P           # class tiles per core (32)
    NCH = 512                 # matmul free-dim per instruction (fp32 max)
    JW = min(4 * NCH, shard)  # psum group width for the main matmul (2048)
    JH = shard // JW          # psum groups per row tile

    Exp = mybir.ActivationFunctionType.Exp
    Ln = mybir.ActivationFunctionType.Ln
    X = mybir.AxisListType.X

    nc = bacc.Bacc("TRN2", target_bir_lowering=False, debug=False,
                   num_devices=n_cores)
    se_d = nc.dram_tensor("se", [n, emb], f32, kind="ExternalInput").ap()
    oe_d = nc.dram_tensor("oe", [shard, emb], f32, kind="ExternalInput").ap()
    pj_d = nc.dram_tensor("proj", [emb, feat], f32, kind="ExternalInput").ap()
    out_d = nc.dram_tensor("out", [n, shard], f32, kind="ExternalOutput").ap()

    with ExitStack() as ctx:
        tc = ctx.enter_context(tile.TileContext(nc))
        const = ctx.enter_context(tc.tile_pool(name="const", bufs=1))
        big = ctx.enter_context(tc.tile_pool(name="big", bufs=1))
        obp = ctx.enter_context(tc.tile_pool(name="obp", bufs=4))
        dram = ctx.enter_context(tc.tile_pool(name="dram", bufs=1, space="DRAM"))

        proj_sb = const.tile([P, feat], f32, name="proj_sb")
        nc.sync.dma_start(out=proj_sb, in_=pj_d)
        projb = const.tile([P, feat], mm_dt, name="projb")
        nc.vector.tensor_copy(out=projb, in_=proj_sb)

        # Inputs in natural layout (partition = row % 128), then cast to bf16,
        # bounce through DRAM, and reload transposed via the DMA xbar engine.
        se_sb = big.tile([P, NT, emb], f32, name="se_sb")
        oe_sb = big.tile([P, JT, emb], f32, name="oe_sb")
        seb = big.tile([P, NT, emb], mm_dt, name="seb")
        oeb = big.tile([P, JT, emb], mm_dt, name="oeb")
        se_sc = dram.tile([n, emb], mm_dt, name="se_sc")
        oe_sc = dram.tile([shard, emb], mm_dt, name="oe_sc")
        seTb = big.tile([P, n], mm_dt, name="seTb")
        oeTb = big.tile([P, shard], mm_dt, name="oeTb")

        ldc = max(1, JT // 4)
        for src_d, sb, cb, sc, tcnt in ((oe_d, oe_sb, oeb, oe_sc, JT),
                                        (se_d, se_sb, seb, se_sc, NT)):
            src_r = src_d.rearrange("(t p) k -> p t k", p=P)
            sc_r = sc.rearrange("(t p) k -> p t k", p=P)
            for g in range(0, tcnt, ldc):
                ge = min(g + ldc, tcnt)
                nc.sync.dma_start(out=sb[:, g:ge, :], in_=src_r[:, g:ge, :])
                nc.vector.tensor_copy(out=cb[:, g:ge, :], in_=sb[:, g:ge, :])
                nc.sync.dma_start(out=sc_r[:, g:ge, :], in_=cb[:, g:ge, :])
        nc.sync.dma_start_transpose(out=oeTb, in_=oe_sc[:])
        nc.sync.dma_start_transpose(out=seTb, in_=se_sc[:])

        # exp of projections, transposed: [f, n] / [f, j], split into FB blocks.
        elTm = [big.tile([P, n], mm_dt, name=f"elTm{fb}") for fb in range(FB)]
        erTm = [big.tile([P, shard], mm_dt, name=f"erTm{fb}") for fb in range(FB)]
        vown = const.tile([P, FB], f32, name="vown")
        vsplit = const.tile([P, FB, 2], mm_dt, name="vsplit")
        vh32 = const.tile([P, FB], f32, name="vh32")
        vlo32 = const.tile([P, FB], f32, name="vlo32")
        z_sb = const.tile([P, NT], f32, name="z_sb")
        zgath = const.tile([P, n_cores, NT], f32, name="zgath")
        Z_sb = const.tile([P, NT], f32, name="Z_sb")
        logz = const.tile([P, NT], f32, name="logz")

        # ---- Phase 1: project, exp, partial normalizer (OE side first so the
        # main matmul's rhs erTm is ready earliest) ----
        with tc.tile_pool(name="ps_pj", bufs=4, space="PSUM") as ps_pj, \
             tc.tile_pool(name="ps_z", bufs=2, space="PSUM") as ps_z:
            for srcT, width, kind in ((oeTb, shard, "r"), (seTb, n, "l")):
                for fb in range(FB):
                    pw = projb[:, fb * P:(fb + 1) * P]
                    dst = erTm[fb] if kind == "r" else elTm[fb]
                    for c0 in range(0, width, NCH):
                        w = min(NCH, width - c0)
                        psl = ps_pj.tile([P, NCH], f32, name="psl", tag="psl")
                        nc.tensor.matmul(psl[:, :w], pw, srcT[:, c0:c0 + w],
                                         start=True, stop=True)
                        nc.scalar.activation(out=dst[:, c0:c0 + w], in_=psl[:, :w],
                                             func=Exp)
                if kind == "r":
                    # v_f = sum over this core's classes of exp(R_jf), fp32.
                    for fb in range(FB):
                        nc.vector.reduce_sum(out=vown[:, fb:fb + 1], in_=erTm[fb],
                                             axis=X)
                    # Split v into bf16 hi + lo so the bf16 matvec keeps ~fp32
                    # precision: v = hi + lo, z = elTm @ [hi, lo].
                    nc.vector.tensor_copy(out=vsplit[:, :, 0], in_=vown)
                    nc.vector.tensor_copy(out=vh32, in_=vsplit[:, :, 0])
                    nc.vector.tensor_sub(vlo32, vown, vh32)
                    nc.vector.tensor_copy(out=vsplit[:, :, 1], in_=vlo32)

            # Per-row partial normalizer z_i = sum_f exp(L_if) * v_f  -> [n]
            for t in range(NT):
                psz = ps_z.tile([P, 2], f32, name="psz", tag="psz")
                for fb in range(FB):
                    nc.tensor.matmul(psz, elTm[fb][:, t * P:(t + 1) * P],
                                     vsplit[:, fb, :],
                                     start=(fb == 0), stop=(fb == FB - 1))
                nc.vector.reduce_sum(out=z_sb[:, t:t + 1], in_=psz, axis=X)

        # ---- AllGather per-core partial normalizers (16 KiB), reduce locally
        # (AllGather has a ~2x lower latency floor than AllReduce).
        cc_in = dram.tile([P, NT], f32, name="cc_in")
        cc_out = dram.tile([P * n_cores, NT], f32, name="cc_out")
        nc.sync.dma_start(out=cc_in, in_=z_sb)
        nc.gpsimd.collective_compute(
            "AllGather", mybir.AluOpType.bypass,
            replica_groups=[list(range(n_cores))],
            ins=[cc_in.opt()], outs=[cc_out.opt()],
        )
        # cc_out rows are [core c][partition p]; bring back as [p, c, t] so the
        # innermost DMA dim stays contiguous, then sum the 8 core blocks.
        nc.sync.dma_start(out=zgath, in_=cc_out.rearrange("(c p) t -> p c t", p=P))
        nc.vector.tensor_add(Z_sb, zgath[:, 0, :], zgath[:, 1, :])
        for c in range(2, n_cores):
            nc.vector.tensor_add(Z_sb, Z_sb, zgath[:, c, :])
        nc.scalar.activation(out=logz, in_=Z_sb, func=Ln)

        # ---- Phase 2: S = exp(L) @ exp(R).T, out = Ln(S) - logZ, store ----
        # The logZ subtraction runs on the (otherwise idle) Vector engine so
        # neither PE nor ACT ever waits on the collective.
        with tc.tile_pool(name="ps_mm", bufs=2, space="PSUM") as ps_mm:
            for t in range(NT):
                ob = obp.tile([P, shard], f32, name="ob", tag="ob")
                for jh in range(JH):
                    ps = ps_mm.tile([P, JW], f32, name="ps", tag="ps")
                    for fb in range(FB):
                        lw = elTm[fb][:, t * P:(t + 1) * P]
                        for c0 in range(0, JW, NCH):
                            nc.tensor.matmul(
                                ps[:, c0:c0 + NCH], lw,
                                erTm[fb][:, jh * JW + c0: jh * JW + c0 + NCH],
                                start=(fb == 0), stop=(fb == FB - 1))
                    osl = ob[:, jh * JW:(jh + 1) * JW]
                    nc.scalar.activation(out=osl, in_=ps, func=Ln)
                    nc.vector.tensor_scalar_sub(osl, osl, logz[:, t:t + 1])
                nc.sync.dma_start(out=out_d[t * P:(t + 1) * P, :], in_=ob)

    nc.compile()
    return nc


def _get_nc():
    if "nc" not in _CACHE:
        _CACHE["nc"] = build_bass()
    return _CACHE["nc"]


def _numpy_fallback(start_emb, output_emb, proj, temp):
    L = start_emb.astype(np.float64) @ proj.astype(np.float64)
    R = output_emb.astype(np.float64) @ proj.astype(np.float64)
    mL = L.max(-1, keepdims=True)
    mR = R.max(-1, keepdims=True)
    S = np.exp(L - mL) @ np.exp(R - mR).T
    logits = (np.log(S) + mL + mR.T) / float(temp)
    m = logits.max(-1, keepdims=True)
    out = logits - m - np.log(np.exp(logits - m).sum(-1, keepdims=True))
    return out.astype(np.float32)


def run_on_hw(start_emb, output_emb, proj, trace=False, **trace_kwargs):
    from concourse.bass_utils import run_bass_kernel_spmd

    nc = _get_nc()
    se = np.ascontiguousarray(start_emb, dtype=np.float32)
    oe = np.ascontiguousarray(output_emb, dtype=np.float32)
    pj = np.ascontiguousarray(proj, dtype=np.float32)
    in_maps = [
        {"se": se, "oe": oe[c * SHARD:(c + 1) * SHARD], "proj": pj}
        for c in range(N_CORES)
    ]
    res = run_bass_kernel_spmd(nc, in_maps, core_ids=list(range(N_CORES)),
                               trace=trace, **trace_kwargs)
    out = np.concatenate([res.results[c]["out"] for c in range(N_CORES)], axis=1)
    return out, res


def kernel(start_emb, output_emb, proj, temp):
    t = float(np.asarray(temp).reshape(-1)[0])
    if t != 1.0:
        return _numpy_fallback(np.asarray(start_emb), np.asarray(output_emb),
                               np.asarray(proj), t)
    out, _ = run_on_hw(start_emb, output_emb, proj, trace=False)
    return out
